# revision 60
# baseline (speedup 1.0000x reference)
"""GQA kernel for Trainium2, tensor-parallel over 8 NeuronCores.

Problem: B=2, S=2048, DIM=2048, 32 q-heads, 8 kv-heads, head_dim=64.
Sharding: core i owns kv-head i and q-heads 4i..4i+3 (Wq/Wk/Wv output-dim
sharded, Wo input-dim sharded). Each core computes a full [B,S,DIM] partial
of the output; the host sums the 8 partials.

Per-core dataflow (all matmul operands bf16, fp32 PSUM accumulation):
  xT (host-pretransposed, [DIM, B*S]) --> QT/KT/VT projections with head-dim
  on partitions; scores computed transposed (S^T[k,q] = KT_blk^T @ QT), exp
  on ScalarE with fused 1/sqrt(hd) scale; AV uses lhsT=[V | 1] so the softmax
  denominator lands in PSUM row 64; normalization via reciprocal_approx_fast
  + rank-1 broadcast matmul; O-proj consumes attention output in its
  [dq, tok] layout.

Scheduling: the ScalarE exp stream is the second-longest engine load, so the
emission order software-pipelines it against TensorE work that does not
depend on it: attention is emitted per (head, 512-query slice) in 2-key-tile
chunks, and after each chunk 1-2 "filler" steps are pulled from a queue of
independent tensor work (next batch's projections, previous slice's O-proj).
This keeps the PE busy during exp latency (so the HAM clock gate stays at
2.4 GHz) and hides the projection/O-proj time entirely inside the attention
phase.
"""
import sys

import numpy as np

sys.path.insert(0, "/opt/trn_rl_repo")

import ml_dtypes
import concourse.bacc as bacc
import concourse.tile as tile
from concourse import mybir
from concourse.masks import make_identity
from concourse import bass_utils

F32 = mybir.dt.float32
BF16 = mybir.dt.bfloat16

B, S, DIM = 2, 2048, 2048
N_HEADS, N_KV = 32, 8
HD = DIM // N_HEADS          # 64
G = N_HEADS // N_KV          # 4 q-heads per kv head (= per core)
DQ = G * HD                  # 256 q-proj cols per core
NCORES = 8
TOKS = B * S                 # 4096
CT = DIM // 128              # 16 contraction tiles
NSL = S // 512               # 4 query slices of 512 per batch
KT_N = S // 128              # 16 key tiles of 128 per batch
SM_SCALE = HD ** -0.5

_CACHE = {}
INTERLEAVE = True


def _build(debug=False):
    nc = bacc.Bacc("TRN2", debug=False, num_devices=NCORES)

    # xT pre-tiled on the host: [b*NSL+tt, partition, ct, 512] so each
    # activation-tile DMA is a contiguous slab
    xT = nc.dram_tensor("xT", [B * NSL, 128, CT, 512], BF16,
                        kind="ExternalInput")
    wq = nc.dram_tensor("wq", [DIM, DQ], BF16, kind="ExternalInput")
    wkv = nc.dram_tensor("wkv", [DIM, 2 * HD], BF16, kind="ExternalInput")
    wo = nc.dram_tensor("wo", [DQ, DIM], BF16, kind="ExternalInput")
    out_p = nc.dram_tensor("out_p", [B, S, DIM], BF16, kind="ExternalOutput")
    if debug:
        dbg = {
            "dbg_kt": nc.dram_tensor("dbg_kt", [64, 512], F32,
                                     kind="ExternalOutput"),
            "dbg_e": nc.dram_tensor("dbg_e", [128, 512], F32,
                                    kind="ExternalOutput"),
            "dbg_den": nc.dram_tensor("dbg_den", [1, 512], F32,
                                      kind="ExternalOutput"),
            "dbg_dinv": nc.dram_tensor("dbg_dinv", [1, 512], F32,
                                       kind="ExternalOutput"),
            "dbg_ao": nc.dram_tensor("dbg_ao", [64, 512], F32,
                                     kind="ExternalOutput"),
        }

    with tile.TileContext(nc) as tc:
        with (
            tc.tile_pool(name="wpool", bufs=1) as wpool,
            tc.tile_pool(name="xpool", bufs=3) as xpool,
            tc.tile_pool(name="actp", bufs=1) as actp,
            tc.tile_pool(name="epool", bufs=3) as epool,
            tc.tile_pool(name="small", bufs=2) as small,
            tc.tile_pool(name="pps", bufs=1, space="PSUM") as pps,
        ):
            # ---- stage weights (chunked so the first proj matmuls can
            # start before the whole weight set has landed) ----
            wkv_sb = wpool.tile([128, CT, 128], BF16)
            wq_sb = wpool.tile([128, CT, 2, 128], BF16)
            wo_sb = wpool.tile([128, 2, 4, 512], BF16)
            ident = wpool.tile([64, 64], BF16)
            make_identity(nc, ident[:])

            # per-batch activation tiles, rotated via tags (bufs=2)
            # qt_p[dt] holds the head pair (2dt, 2dt+1) interleaved on the
            # middle axis so one N=1024 matmul scores both heads at once.
            def batch_tiles():
                qt_p = [
                    actp.tile([64, 2, S], BF16, tag=f"qt{d}", name=f"qt{d}",
                              bufs=2)
                    for d in range(2)
                ]
                kt = actp.tile([64, S], BF16, tag="kt", bufs=2)
                vt = actp.tile([64, S], BF16, tag="vt", bufs=2)
                v1 = actp.tile([128, KT_N, 65], BF16, tag="v1", bufs=2)
                ao2 = [
                    actp.tile([128, S], BF16, tag=f"ao{d}", name=f"ao{d}", bufs=2)
                    for d in range(2)
                ]
                return qt_p, kt, vt, v1, ao2

            dma_rr = [0]
            dma_engs = (nc.sync, nc.gpsimd)

            def next_dma():
                e = dma_engs[dma_rr[0] % 2]
                dma_rr[0] += 1
                return e

            # xc quarters: [128, 4, 512] each; 16 per batch, never reused
            # within a batch (so late Q-unit consumers stay race-free)
            xcq = {}

            def load_xc(b, tt):
                # batch 0 loads run while ScalarE is idle (prologue), so its
                # queue can help; batch 1 loads must stay off ScalarE (exps)
                if (b, tt) in xcq:
                    return
                engs = (nc.sync, nc.gpsimd, nc.scalar) if b == 0 else \
                    (nc.sync, nc.gpsimd)
                q4 = []
                for k in range(4):
                    xq = xpool.tile([128, 4, 512], BF16, tag="xc", bufs=20,
                                    name="xcq")
                    engs[k % len(engs)].dma_start(
                        xq[:], xT.ap()[b * NSL + tt, :, k * 4:(k + 1) * 4, :])
                    q4.append(xq)
                xcq[(b, tt)] = q4

            def m_unit(b, tt, m, tiles):
                """One projection output tile: 16 accumulating matmuls."""
                qt_p, kt, vt, v1, ao2 = tiles
                qs_ = slice(tt * 512, (tt + 1) * 512)
                ps = pps.tile([128, 512], F32, tag="fil", bufs=2,
                              name=f"proj{m}")
                for ci0 in range(0, CT, 2):
                    for ci in (ci0, ci0 + 1):
                        w_ap = (wq_sb[:, ci, m, :] if m < 2
                                else wkv_sb[:, ci, :])
                        nc.tensor.matmul(ps[:], w_ap,
                                         xcq[(b, tt)][ci // 4][:, ci % 4, :],
                                         start=ci == 0, stop=ci == CT - 1)
                    yield
                if m < 2:
                    nc.vector.tensor_copy(qt_p[m][:, 0, qs_], ps[0:64, :])
                    nc.vector.tensor_copy(qt_p[m][:, 1, qs_], ps[64:128, :])
                else:
                    nc.vector.tensor_copy(kt[:, qs_], ps[0:64, :])
                    nc.vector.tensor_copy(vt[:, qs_], ps[64:128, :])
                yield

            def proj_steps(b, tiles):
                """Full projection of batch b: per token tile kv, q0, q1."""
                qt_p, kt, vt, v1, ao2 = tiles
                nc.vector.memset(v1[:, :, 64:65], 1.0)
                load_xc(b, 0)
                for tt in range(NSL):
                    if tt + 1 < NSL:
                        load_xc(b, tt + 1)
                        yield
                    for m in (2, 0, 1):
                        yield from m_unit(b, tt, m, tiles)
                    ptr = pps.tile([128, 4, 64], BF16, tag="fil", bufs=2,
                                   name="ptr")
                    for j in range(4):
                        ki = tt * 4 + j
                        nc.tensor.transpose(
                            ptr[:, j, :], vt[:, ki * 128:(ki + 1) * 128],
                            ident[:]
                        )
                    yield
                    nc.vector.tensor_copy(v1[:, tt * 4:(tt + 1) * 4, 0:64],
                                          ptr[:])
                    yield

            def oproj_steps(b, s, ao2):
                """Generator: O-projection + store of 512-token slice s."""
                # defer the first matmul a few pulls so it doesn't reach the
                # PE queue before the slice's ao2 muls have cleared VectorE
                for _ in range(3):
                    yield
                tail = b == B - 1 and s == NSL - 1
                cp = 0
                for t2 in range(s * 4, (s + 1) * 4):
                    ts_ = slice(t2 * 128, (t2 + 1) * 128)
                    osb = epool.tile([128, 4, 512], BF16, tag="osb", bufs=2)
                    for half in range(2):
                        for nt in range(2):
                            # in the drain phase attention is over, so the
                            # sc banks are free — use them for extra depth
                            po_tag = ("sc" if (tail and cp % 2 == 0)
                                      else "fil")
                            po = pps.tile([128, 512], F32, tag=po_tag,
                                          bufs=2, name="po")
                            for dt in range(2):
                                nc.tensor.matmul(
                                    po[:], ao2[dt][:, ts_],
                                    wo_sb[:, dt, half * 2 + nt, :],
                                    start=dt == 0, stop=dt == 1,
                                )
                            yield
                            # on the final slice ScalarE is done with exps;
                            # let it take half the PSUM->SBUF copies
                            if tail and cp % 2:
                                nc.scalar.copy(osb[:, half * 2 + nt, :],
                                               po[:])
                            else:
                                nc.vector.tensor_copy(
                                    osb[:, half * 2 + nt, :], po[:])
                            cp += 1
                            yield
                    if tail and t2 >= s * 4 + 2:
                        # final stores: split across two queues to shorten
                        # the post-compute drain
                        nc.sync.dma_start(out_p.ap()[b, ts_, 0:1024],
                                          osb[:, 0:2, :])
                        nc.gpsimd.dma_start(out_p.ap()[b, ts_, 1024:2048],
                                            osb[:, 2:4, :])
                    else:
                        next_dma().dma_start(out_p.ap()[b, ts_, :], osb[:])
                    yield

            # ---- filler machinery ----
            filler = []  # list of generators, head consumed first

            def pull(n, force=False):
                if not INTERLEAVE and not force:
                    return
                while n > 0 and filler:
                    try:
                        next(filler[0])
                        n -= 1
                    except StopIteration:
                        filler.pop(0)

            # ---- input staging, in consumption order: wkv chunk 0 + the
            # first x tile feed the first kv matmuls within ~2us; then the
            # rest of wkv/wq; wo (needed ~80us in) goes last ----
            wkv_r = wkv.ap().rearrange("(ct p) d -> p ct d", p=128)
            nc.scalar.dma_start(wkv_sb[:, 0:4], wkv_r[:, 0:4])
            load_xc(0, 0)
            for i, c4 in enumerate(range(4, CT, 4)):
                (nc.scalar, nc.sync, nc.gpsimd)[i % 3].dma_start(
                    wkv_sb[:, c4:c4 + 4], wkv_r[:, c4:c4 + 4])
            wq_r = wq.ap().rearrange("(ct p) (dt m) -> p ct dt m", p=128,
                                     m=128)
            for i, c4 in enumerate(range(0, CT, 4)):
                (nc.sync, nc.gpsimd, nc.scalar)[i % 3].dma_start(
                    wq_sb[:, c4:c4 + 4], wq_r[:, c4:c4 + 4])
            nc.scalar.dma_start(
                wo_sb[:],
                wo.ap().rearrange("(dt p) (nt n) -> p dt nt n", p=128, n=512)
            )

            # ---- prologue: batch-0 projections, emitted eagerly ----
            tiles = [batch_tiles(), None]
            for _ in proj_steps(0, tiles[0]):
                pass

            # ---- main: per batch, per query-slice, per head ----
            for b in range(B):
                qt_p, kt, vt, v1, ao2 = tiles[b]
                if b + 1 < B:
                    tiles[b + 1] = batch_tiles()
                    filler.append(proj_steps(b + 1, tiles[b + 1]))
                for s in range(NSL):
                    ss_ = slice(s * 512, (s + 1) * 512)
                    for g in range(G):
                        av = pps.tile([128, 512], F32, tag="av", bufs=2,
                                      name="av")
                        prev = None
                        for kc in range(8):
                            sc = pps.tile([128, 2, 512], F32, tag="sc", bufs=2,
                                          name="sc")
                            for j in range(2):
                                ki = kc * 2 + j
                                nc.tensor.matmul(
                                    sc[:, j, :],
                                    kt[:, ki * 128:(ki + 1) * 128],
                                    qt_p[g // 2][:, g % 2, ss_],
                                    start=True, stop=True,
                                )
                            e = epool.tile([128, 2, 512], BF16, tag="e",
                                           bufs=4)
                            nc.scalar.activation(
                                e[:], sc[:],
                                mybir.ActivationFunctionType.Exp,
                                scale=SM_SCALE,
                            )
                            if debug and b == 0 and s == 0 and g == 0 \
                                    and kc == 0:
                                t_e = small.tile([128, 512], F32, tag="dbge",
                                                 bufs=1)
                                nc.vector.tensor_copy(t_e[:], e[:, 0, :])
                                nc.sync.dma_start(dbg["dbg_e"].ap(), t_e[:])
                            if prev is not None:
                                pe, pkc = prev
                                for j in range(2):
                                    nc.tensor.matmul(
                                        av[0:65, :], v1[:, pkc * 2 + j, :],
                                        pe[:, j, :],
                                        start=(pkc == 0 and j == 0),
                                        stop=False,
                                    )
                            pull(2)
                            prev = (e, kc)
                        pe, pkc = prev
                        for j in range(2):
                            nc.tensor.matmul(
                                av[0:65, :], v1[:, pkc * 2 + j, :],
                                pe[:, j, :],
                                start=False, stop=(j == 1),
                            )
                        pull(1)
                        # normalization: den -> 1/den on partition 0, then
                        # GpSimd partition-broadcast to 64 rows, one mul
                        den_sb = small.tile([1, 512], F32, tag="densb",
                                            bufs=2)
                        nc.vector.tensor_copy(den_sb[:], av[64:65, :])
                        den_inv = small.tile([1, 512], F32, tag="deninv",
                                             bufs=2)
                        nc.vector.reciprocal_approx_fast(den_inv[:],
                                                         den_sb[:])
                        bc_sb = small.tile([64, 512], F32, tag="bc", bufs=2)
                        nc.gpsimd.partition_broadcast(bc_sb[:], den_inv[:])
                        nc.vector.tensor_mul(
                            ao2[g // 2][(g % 2) * 64:(g % 2) * 64 + 64, ss_],
                            av[0:64, :], bc_sb[:],
                        )
                        if debug and b == 0 and s == 0 and g == 0:
                            t_kt = small.tile([64, 512], F32, tag="dbgkt",
                                              bufs=1)
                            nc.vector.tensor_copy(t_kt[:], kt[:, 0:512])
                            nc.sync.dma_start(dbg["dbg_kt"].ap(), t_kt[:])
                            t_den = small.tile([1, 512], F32, tag="dbgden",
                                               bufs=1)
                            nc.vector.tensor_copy(t_den[:], av[64:65, :])
                            nc.sync.dma_start(dbg["dbg_den"].ap(), t_den[:])
                            nc.sync.dma_start(dbg["dbg_dinv"].ap(),
                                              den_inv[:])
                            t_ao = small.tile([64, 512], F32, tag="dbgao",
                                              bufs=1)
                            nc.vector.tensor_copy(
                                t_ao[:], ao2[0][0:64, 0:512])
                            nc.sync.dma_start(dbg["dbg_ao"].ap(), t_ao[:])
                    filler.append(oproj_steps(b, s, ao2))
                    if not INTERLEAVE:
                        pull(1 << 30, force=True)
            # drain remaining filler (last slice's O-proj)
            pull(1 << 30, force=True)

    nc.compile()
    return nc


def _get_nc():
    if "nc" not in _CACHE:
        _CACHE["nc"] = _build()
    return _CACHE["nc"]


def kernel(x, Wq, Wk, Wv, Wo, _trace=False):
    nc = _get_nc()
    bf = ml_dtypes.bfloat16
    xT = np.ascontiguousarray(
        np.asarray(x, np.float32)
        .reshape(B, NSL, 512, CT, 128).transpose(0, 1, 4, 3, 2)
    ).astype(bf).reshape(B * NSL, 128, CT, 512)
    Wq = np.asarray(Wq, np.float32)
    Wk = np.asarray(Wk, np.float32)
    Wv = np.asarray(Wv, np.float32)
    Wo = np.asarray(Wo, np.float32)

    in_maps = []
    for c in range(NCORES):
        wq_c = Wq[:, c * DQ:(c + 1) * DQ].astype(bf)
        wkv_c = np.concatenate(
            [Wk[:, c * HD:(c + 1) * HD], Wv[:, c * HD:(c + 1) * HD]], axis=1
        ).astype(bf)
        wo_c = Wo[c * DQ:(c + 1) * DQ, :].astype(bf)
        in_maps.append({"xT": xT, "wq": np.ascontiguousarray(wq_c),
                        "wkv": np.ascontiguousarray(wkv_c),
                        "wo": np.ascontiguousarray(wo_c)})

    res = bass_utils.run_bass_kernel_spmd(
        nc, in_maps, core_ids=list(range(NCORES)), trace=_trace
    )
    out = res.results[0]["out_p"].astype(np.float64)
    for c in range(1, NCORES):
        out += res.results[c]["out_p"].astype(np.float64)
    if _trace:
        kernel.last_exec_time_ns = res.exec_time_ns
        kernel.last_results = res
    return out.astype(np.float32)


kernel.last_exec_time_ns = None


def kernel_debug(x, Wq, Wk, Wv, Wo):
    if "ncd" not in _CACHE:
        _CACHE["ncd"] = _build(debug=True)
    nc = _CACHE["ncd"]
    bf = ml_dtypes.bfloat16
    xT = np.ascontiguousarray(
        np.asarray(x, np.float32)
        .reshape(B, NSL, 512, CT, 128).transpose(0, 1, 4, 3, 2)
    ).astype(bf).reshape(B * NSL, 128, CT, 512)
    Wq = np.asarray(Wq, np.float32)
    Wk = np.asarray(Wk, np.float32)
    Wv = np.asarray(Wv, np.float32)
    Wo = np.asarray(Wo, np.float32)
    in_maps = []
    for c in range(NCORES):
        wq_c = Wq[:, c * DQ:(c + 1) * DQ].astype(bf)
        wkv_c = np.concatenate(
            [Wk[:, c * HD:(c + 1) * HD], Wv[:, c * HD:(c + 1) * HD]], axis=1
        ).astype(bf)
        wo_c = Wo[c * DQ:(c + 1) * DQ, :].astype(bf)
        in_maps.append({"xT": xT, "wq": np.ascontiguousarray(wq_c),
                        "wkv": np.ascontiguousarray(wkv_c),
                        "wo": np.ascontiguousarray(wo_c)})
    res = bass_utils.run_bass_kernel_spmd(
        nc, in_maps, core_ids=list(range(NCORES))
    )
    return {k: np.asarray(v, np.float32)
            for k, v in res.results[0].items() if k.startswith("dbg")}


# revision 61
# speedup vs baseline: 1.0169x; 1.0169x over previous
"""GQA kernel for Trainium2, tensor-parallel over 8 NeuronCores.

Problem: B=2, S=2048, DIM=2048, 32 q-heads, 8 kv-heads, head_dim=64.
Sharding: core i owns kv-head i and q-heads 4i..4i+3 (Wq/Wk/Wv output-dim
sharded, Wo input-dim sharded). Each core computes a full [B,S,DIM] partial
of the output; the host sums the 8 partials.

Per-core dataflow (all matmul operands bf16, fp32 PSUM accumulation):
  xT (host-pretransposed, [DIM, B*S]) --> QT/KT/VT projections with head-dim
  on partitions; scores computed transposed (S^T[k,q] = KT_blk^T @ QT), exp
  on ScalarE with fused 1/sqrt(hd) scale; AV uses lhsT=[V | 1] so the softmax
  denominator lands in PSUM row 64; normalization via reciprocal_approx_fast
  + rank-1 broadcast matmul; O-proj consumes attention output in its
  [dq, tok] layout.

Scheduling: the ScalarE exp stream is the second-longest engine load, so the
emission order software-pipelines it against TensorE work that does not
depend on it: attention is emitted per (head, 512-query slice) in 2-key-tile
chunks, and after each chunk 1-2 "filler" steps are pulled from a queue of
independent tensor work (next batch's projections, previous slice's O-proj).
This keeps the PE busy during exp latency (so the HAM clock gate stays at
2.4 GHz) and hides the projection/O-proj time entirely inside the attention
phase.
"""
import sys

import numpy as np

sys.path.insert(0, "/opt/trn_rl_repo")

import ml_dtypes
import concourse.bacc as bacc
import concourse.tile as tile
from concourse import mybir
from concourse.masks import make_identity
from concourse import bass_utils

F32 = mybir.dt.float32
BF16 = mybir.dt.bfloat16

B, S, DIM = 2, 2048, 2048
N_HEADS, N_KV = 32, 8
HD = DIM // N_HEADS          # 64
G = N_HEADS // N_KV          # 4 q-heads per kv head (= per core)
DQ = G * HD                  # 256 q-proj cols per core
NCORES = 8
TOKS = B * S                 # 4096
CT = DIM // 128              # 16 contraction tiles
NSL = S // 512               # 4 query slices of 512 per batch
KT_N = S // 128              # 16 key tiles of 128 per batch
SM_SCALE = HD ** -0.5

_CACHE = {}
INTERLEAVE = True


def _build(debug=False):
    nc = bacc.Bacc("TRN2", debug=False, num_devices=NCORES)

    # xT pre-tiled on the host: [b*NSL+tt, partition, ct, 512] so each
    # activation-tile DMA is a contiguous slab
    xT = nc.dram_tensor("xT", [B * NSL, 128, CT, 512], BF16,
                        kind="ExternalInput")
    wq = nc.dram_tensor("wq", [DIM, DQ], BF16, kind="ExternalInput")
    wkv = nc.dram_tensor("wkv", [DIM, 2 * HD], BF16, kind="ExternalInput")
    wo = nc.dram_tensor("wo", [DQ, DIM], BF16, kind="ExternalInput")
    out_p = nc.dram_tensor("out_p", [B, S, DIM], BF16, kind="ExternalOutput")
    if debug:
        dbg = {
            "dbg_kt": nc.dram_tensor("dbg_kt", [64, 512], F32,
                                     kind="ExternalOutput"),
            "dbg_e": nc.dram_tensor("dbg_e", [128, 512], F32,
                                    kind="ExternalOutput"),
            "dbg_den": nc.dram_tensor("dbg_den", [1, 512], F32,
                                      kind="ExternalOutput"),
            "dbg_dinv": nc.dram_tensor("dbg_dinv", [1, 512], F32,
                                       kind="ExternalOutput"),
            "dbg_ao": nc.dram_tensor("dbg_ao", [64, 512], F32,
                                     kind="ExternalOutput"),
        }

    with tile.TileContext(nc) as tc:
        with (
            tc.tile_pool(name="wpool", bufs=1) as wpool,
            tc.tile_pool(name="xpool", bufs=3) as xpool,
            tc.tile_pool(name="actp", bufs=1) as actp,
            tc.tile_pool(name="epool", bufs=3) as epool,
            tc.tile_pool(name="small", bufs=2) as small,
            tc.tile_pool(name="pps", bufs=1, space="PSUM") as pps,
        ):
            # ---- stage weights (chunked so the first proj matmuls can
            # start before the whole weight set has landed) ----
            wkv_sb = wpool.tile([128, CT, 128], BF16)
            wq_sb = wpool.tile([128, CT, 2, 128], BF16)
            wo_sb = wpool.tile([128, 2, 4, 512], BF16)
            ident = wpool.tile([64, 64], BF16)
            make_identity(nc, ident[:])

            # per-batch activation tiles, rotated via tags (bufs=2)
            # qt_p[dt] holds the head pair (2dt, 2dt+1) interleaved on the
            # middle axis so one N=1024 matmul scores both heads at once.
            def batch_tiles():
                qt_p = [
                    actp.tile([64, 2, S], BF16, tag=f"qt{d}", name=f"qt{d}",
                              bufs=2)
                    for d in range(2)
                ]
                kt = actp.tile([64, S], BF16, tag="kt", bufs=2)
                vt = actp.tile([64, S], BF16, tag="vt", bufs=2)
                v1 = actp.tile([128, KT_N, 65], BF16, tag="v1", bufs=2)
                ao2 = [
                    actp.tile([128, S], BF16, tag=f"ao{d}", name=f"ao{d}", bufs=2)
                    for d in range(2)
                ]
                return qt_p, kt, vt, v1, ao2

            dma_rr = [0]
            dma_engs = (nc.sync, nc.gpsimd)

            def next_dma():
                e = dma_engs[dma_rr[0] % 2]
                dma_rr[0] += 1
                return e

            # xc quarters: [128, 4, 512] each; 16 per batch, never reused
            # within a batch (so late Q-unit consumers stay race-free)
            xcq = {}

            def load_xc(b, tt):
                # batch 0 loads run while ScalarE is idle (prologue), so its
                # queue can help; batch 1 loads must stay off ScalarE (exps)
                if (b, tt) in xcq:
                    return
                engs = (nc.sync, nc.gpsimd, nc.scalar) if b == 0 else \
                    (nc.sync, nc.gpsimd)
                q4 = []
                for k in range(4):
                    xq = xpool.tile([128, 4, 512], BF16, tag="xc", bufs=20,
                                    name="xcq")
                    engs[k % len(engs)].dma_start(
                        xq[:], xT.ap()[b * NSL + tt, :, k * 4:(k + 1) * 4, :])
                    q4.append(xq)
                xcq[(b, tt)] = q4

            def m_unit(b, tt, m, tiles):
                """One projection output tile: 16 accumulating matmuls."""
                qt_p, kt, vt, v1, ao2 = tiles
                qs_ = slice(tt * 512, (tt + 1) * 512)
                ps = pps.tile([128, 512], F32, tag="fil", bufs=2,
                              name=f"proj{m}")
                for ci0 in range(0, CT, 2):
                    for ci in (ci0, ci0 + 1):
                        w_ap = (wq_sb[:, ci, m, :] if m < 2
                                else wkv_sb[:, ci, :])
                        nc.tensor.matmul(ps[:], w_ap,
                                         xcq[(b, tt)][ci // 4][:, ci % 4, :],
                                         start=ci == 0, stop=ci == CT - 1)
                    yield
                if m < 2:
                    nc.vector.tensor_copy(qt_p[m][:, 0, qs_], ps[0:64, :])
                    nc.vector.tensor_copy(qt_p[m][:, 1, qs_], ps[64:128, :])
                else:
                    nc.vector.tensor_copy(kt[:, qs_], ps[0:64, :])
                    nc.vector.tensor_copy(vt[:, qs_], ps[64:128, :])
                yield

            def proj_steps(b, tiles):
                """Full projection of batch b: per token tile kv, q0, q1."""
                qt_p, kt, vt, v1, ao2 = tiles
                nc.vector.memset(v1[:, :, 64:65], 1.0)
                load_xc(b, 0)
                for tt in range(NSL):
                    if tt + 1 < NSL:
                        load_xc(b, tt + 1)
                        yield
                    for m in (2, 0, 1):
                        yield from m_unit(b, tt, m, tiles)
                    ptr = pps.tile([128, 4, 64], BF16, tag="fil", bufs=2,
                                   name="ptr")
                    for j in range(4):
                        ki = tt * 4 + j
                        nc.tensor.transpose(
                            ptr[:, j, :], vt[:, ki * 128:(ki + 1) * 128],
                            ident[:]
                        )
                    yield
                    nc.vector.tensor_copy(v1[:, tt * 4:(tt + 1) * 4, 0:64],
                                          ptr[:])
                    yield

            def oproj_steps(b, s, ao2):
                """Generator: O-projection + store of 512-token slice s."""
                # defer the first matmul a few pulls so it doesn't reach the
                # PE queue before the slice's ao2 muls have cleared VectorE
                for _ in range(3):
                    yield
                tail = b == B - 1 and s == NSL - 1
                cp = 0
                for t2 in range(s * 4, (s + 1) * 4):
                    ts_ = slice(t2 * 128, (t2 + 1) * 128)
                    osb = epool.tile([128, 4, 512], BF16, tag="osb", bufs=2)
                    for half in range(2):
                        for nt in range(2):
                            # in the drain phase attention is over, so the
                            # sc banks are free — use them for extra depth
                            po_tag = ("sc" if (tail and cp % 2 == 0)
                                      else "fil")
                            po = pps.tile([128, 512], F32, tag=po_tag,
                                          bufs=2, name="po")
                            for dt in range(2):
                                nc.tensor.matmul(
                                    po[:], ao2[dt][:, ts_],
                                    wo_sb[:, dt, half * 2 + nt, :],
                                    start=dt == 0, stop=dt == 1,
                                )
                            yield
                            # on the final slice ScalarE is done with exps;
                            # let it take half the PSUM->SBUF copies
                            if tail and cp % 2:
                                nc.scalar.copy(osb[:, half * 2 + nt, :],
                                               po[:])
                            else:
                                nc.vector.tensor_copy(
                                    osb[:, half * 2 + nt, :], po[:])
                            cp += 1
                            yield
                    if tail and t2 >= s * 4 + 2:
                        # final stores: split across two queues to shorten
                        # the post-compute drain
                        nc.sync.dma_start(out_p.ap()[b, ts_, 0:1024],
                                          osb[:, 0:2, :])
                        nc.gpsimd.dma_start(out_p.ap()[b, ts_, 1024:2048],
                                            osb[:, 2:4, :])
                    else:
                        next_dma().dma_start(out_p.ap()[b, ts_, :], osb[:])
                    yield

            # ---- filler machinery ----
            filler = []  # list of generators, head consumed first

            def pull(n, force=False):
                if not INTERLEAVE and not force:
                    return
                while n > 0 and filler:
                    try:
                        next(filler[0])
                        n -= 1
                    except StopIteration:
                        filler.pop(0)

            # ---- input staging, in consumption order: wkv chunk 0 + the
            # first x tile feed the first kv matmuls within ~2us; then the
            # rest of wkv/wq; wo (needed ~80us in) goes last ----
            wkv_r = wkv.ap().rearrange("(ct p) d -> p ct d", p=128)
            nc.scalar.dma_start(wkv_sb[:, 0:4], wkv_r[:, 0:4])
            load_xc(0, 0)
            for i, c4 in enumerate(range(4, CT, 4)):
                (nc.scalar, nc.sync, nc.gpsimd)[i % 3].dma_start(
                    wkv_sb[:, c4:c4 + 4], wkv_r[:, c4:c4 + 4])
            wq_r = wq.ap().rearrange("(ct p) (dt m) -> p ct dt m", p=128,
                                     m=128)
            for i, c4 in enumerate(range(0, CT, 4)):
                (nc.sync, nc.gpsimd, nc.scalar)[i % 3].dma_start(
                    wq_sb[:, c4:c4 + 4], wq_r[:, c4:c4 + 4])
            load_xc(0, 1)

            # ---- prologue: batch-0 projections, emitted eagerly ----
            tiles = [batch_tiles(), None]
            for _ in proj_steps(0, tiles[0]):
                pass
            # wo is first needed by O-proj ~80us in; load it after all of
            # batch 0's x tiles so it never delays them in a DMA queue
            nc.scalar.dma_start(
                wo_sb[:],
                wo.ap().rearrange("(dt p) (nt n) -> p dt nt n", p=128, n=512)
            )

            # ---- main: per batch, per query-slice, per head ----
            for b in range(B):
                qt_p, kt, vt, v1, ao2 = tiles[b]
                if b + 1 < B:
                    tiles[b + 1] = batch_tiles()
                    filler.append(proj_steps(b + 1, tiles[b + 1]))
                for s in range(NSL):
                    ss_ = slice(s * 512, (s + 1) * 512)
                    for g in range(G):
                        av = pps.tile([128, 512], F32, tag="av", bufs=2,
                                      name="av")
                        prev = None
                        for kc in range(8):
                            sc = pps.tile([128, 2, 512], F32, tag="sc", bufs=2,
                                          name="sc")
                            for j in range(2):
                                ki = kc * 2 + j
                                nc.tensor.matmul(
                                    sc[:, j, :],
                                    kt[:, ki * 128:(ki + 1) * 128],
                                    qt_p[g // 2][:, g % 2, ss_],
                                    start=True, stop=True,
                                )
                            e = epool.tile([128, 2, 512], BF16, tag="e",
                                           bufs=4)
                            nc.scalar.activation(
                                e[:], sc[:],
                                mybir.ActivationFunctionType.Exp,
                                scale=SM_SCALE,
                            )
                            if debug and b == 0 and s == 0 and g == 0 \
                                    and kc == 0:
                                t_e = small.tile([128, 512], F32, tag="dbge",
                                                 bufs=1)
                                nc.vector.tensor_copy(t_e[:], e[:, 0, :])
                                nc.sync.dma_start(dbg["dbg_e"].ap(), t_e[:])
                            if prev is not None:
                                pe, pkc = prev
                                for j in range(2):
                                    nc.tensor.matmul(
                                        av[0:65, :], v1[:, pkc * 2 + j, :],
                                        pe[:, j, :],
                                        start=(pkc == 0 and j == 0),
                                        stop=False,
                                    )
                            pull(2)
                            prev = (e, kc)
                        pe, pkc = prev
                        for j in range(2):
                            nc.tensor.matmul(
                                av[0:65, :], v1[:, pkc * 2 + j, :],
                                pe[:, j, :],
                                start=False, stop=(j == 1),
                            )
                        pull(1)
                        # normalization: den -> 1/den on partition 0, then
                        # GpSimd partition-broadcast to 64 rows, one mul
                        den_sb = small.tile([1, 512], F32, tag="densb",
                                            bufs=2)
                        nc.vector.tensor_copy(den_sb[:], av[64:65, :])
                        den_inv = small.tile([1, 512], F32, tag="deninv",
                                             bufs=2)
                        nc.vector.reciprocal_approx_fast(den_inv[:],
                                                         den_sb[:])
                        bc_sb = small.tile([64, 512], F32, tag="bc", bufs=2)
                        nc.gpsimd.partition_broadcast(bc_sb[:], den_inv[:])
                        nc.vector.tensor_mul(
                            ao2[g // 2][(g % 2) * 64:(g % 2) * 64 + 64, ss_],
                            av[0:64, :], bc_sb[:],
                        )
                        if debug and b == 0 and s == 0 and g == 0:
                            t_kt = small.tile([64, 512], F32, tag="dbgkt",
                                              bufs=1)
                            nc.vector.tensor_copy(t_kt[:], kt[:, 0:512])
                            nc.sync.dma_start(dbg["dbg_kt"].ap(), t_kt[:])
                            t_den = small.tile([1, 512], F32, tag="dbgden",
                                               bufs=1)
                            nc.vector.tensor_copy(t_den[:], av[64:65, :])
                            nc.sync.dma_start(dbg["dbg_den"].ap(), t_den[:])
                            nc.sync.dma_start(dbg["dbg_dinv"].ap(),
                                              den_inv[:])
                            t_ao = small.tile([64, 512], F32, tag="dbgao",
                                              bufs=1)
                            nc.vector.tensor_copy(
                                t_ao[:], ao2[0][0:64, 0:512])
                            nc.sync.dma_start(dbg["dbg_ao"].ap(), t_ao[:])
                    filler.append(oproj_steps(b, s, ao2))
                    if not INTERLEAVE:
                        pull(1 << 30, force=True)
            # drain remaining filler (last slice's O-proj)
            pull(1 << 30, force=True)

    nc.compile()
    return nc


def _get_nc():
    if "nc" not in _CACHE:
        _CACHE["nc"] = _build()
    return _CACHE["nc"]


def kernel(x, Wq, Wk, Wv, Wo, _trace=False):
    nc = _get_nc()
    bf = ml_dtypes.bfloat16
    xT = np.ascontiguousarray(
        np.asarray(x, np.float32)
        .reshape(B, NSL, 512, CT, 128).transpose(0, 1, 4, 3, 2)
    ).astype(bf).reshape(B * NSL, 128, CT, 512)
    Wq = np.asarray(Wq, np.float32)
    Wk = np.asarray(Wk, np.float32)
    Wv = np.asarray(Wv, np.float32)
    Wo = np.asarray(Wo, np.float32)

    in_maps = []
    for c in range(NCORES):
        wq_c = Wq[:, c * DQ:(c + 1) * DQ].astype(bf)
        wkv_c = np.concatenate(
            [Wk[:, c * HD:(c + 1) * HD], Wv[:, c * HD:(c + 1) * HD]], axis=1
        ).astype(bf)
        wo_c = Wo[c * DQ:(c + 1) * DQ, :].astype(bf)
        in_maps.append({"xT": xT, "wq": np.ascontiguousarray(wq_c),
                        "wkv": np.ascontiguousarray(wkv_c),
                        "wo": np.ascontiguousarray(wo_c)})

    res = bass_utils.run_bass_kernel_spmd(
        nc, in_maps, core_ids=list(range(NCORES)), trace=_trace
    )
    out = res.results[0]["out_p"].astype(np.float64)
    for c in range(1, NCORES):
        out += res.results[c]["out_p"].astype(np.float64)
    if _trace:
        kernel.last_exec_time_ns = res.exec_time_ns
        kernel.last_results = res
    return out.astype(np.float32)


kernel.last_exec_time_ns = None


def kernel_debug(x, Wq, Wk, Wv, Wo):
    if "ncd" not in _CACHE:
        _CACHE["ncd"] = _build(debug=True)
    nc = _CACHE["ncd"]
    bf = ml_dtypes.bfloat16
    xT = np.ascontiguousarray(
        np.asarray(x, np.float32)
        .reshape(B, NSL, 512, CT, 128).transpose(0, 1, 4, 3, 2)
    ).astype(bf).reshape(B * NSL, 128, CT, 512)
    Wq = np.asarray(Wq, np.float32)
    Wk = np.asarray(Wk, np.float32)
    Wv = np.asarray(Wv, np.float32)
    Wo = np.asarray(Wo, np.float32)
    in_maps = []
    for c in range(NCORES):
        wq_c = Wq[:, c * DQ:(c + 1) * DQ].astype(bf)
        wkv_c = np.concatenate(
            [Wk[:, c * HD:(c + 1) * HD], Wv[:, c * HD:(c + 1) * HD]], axis=1
        ).astype(bf)
        wo_c = Wo[c * DQ:(c + 1) * DQ, :].astype(bf)
        in_maps.append({"xT": xT, "wq": np.ascontiguousarray(wq_c),
                        "wkv": np.ascontiguousarray(wkv_c),
                        "wo": np.ascontiguousarray(wo_c)})
    res = bass_utils.run_bass_kernel_spmd(
        nc, in_maps, core_ids=list(range(NCORES))
    )
    return {k: np.asarray(v, np.float32)
            for k, v in res.results[0].items() if k.startswith("dbg")}
